# revision 15
# baseline (speedup 1.0000x reference)
"""Trainium2 Bass kernel for the prototype-bank scatter-mean EMA update
(nn_Bank): class-sharded sorted-segment reduction across 8 NeuronCores.

Host (index/layout work only; all FP reduction arithmetic is on device):
  * argsort labels; assign each class to one core (greedy token balance,
    <=128 classes/core); concatenate each core's class segments, zero-
    padding every class to a multiple of GTOK tokens so each GTOK-token
    "block" is single-class.
  * feature blocks are packed feature-major [65, GTOK] in bf16: rows
    0..63 = the block's GTOK token features (transposed), row 64 = a
    1.0/0.0 valid-token indicator (so the same reduction that produces
    block feature sums also produces block counts).
  * per-core metadata: block -> local-class id; prototype rows for the
    core's classes; an iota row table.

Device, per core (T tiles; tile = 128 blocks = 128*GTOK tokens):
  1. Stream feature tiles [128, 65*GTOK] bf16 HBM->SBUF (sync queue).
  2. Block sums: tiles alternate between DVE tensor_reduce (true sums)
     and Pool avg-pool (sums/GTOK) -> rhs [128, 65] bf16.
  3. DVE builds one-hot lhsT [128 blocks, 128 local classes] bf16 via
     (iota == cls) * scale, scale = GTOK for Pool tiles (undoes the avg)
     and 1 for DVE tiles.
  4. PE: psum[cls, 65] += oh^T @ rhs, PSUM-accumulated over all T tiles
     -> per-class feature sums (cols 0..63) and counts (col 64).
  5. Blend: means = sums/max(cnt,1); out = proto + s*(means-proto) with
     s = present * (0.1 + 0.9*use_new)  [step>warmup branch].
  6. DMA out [128, D]; host scatters per-core rows back to [1000, 64].

No collective: every class is fully owned by one core.
"""

import numpy as np

import concourse.bacc as bacc
import concourse.bass as bass
import concourse.mybir as mybir
from concourse import bass_utils

C = 1000
D = 64
E = D + 1            # feature dims + count indicator (metadata only)
P = 128
GTOK = 32            # tokens per block (class padding granularity)
LAM = 0.9
WARMUP_STEP = 1000
N_CORES = 8
NB = 8               # feature tile buffers
NR = 4               # rhs / one-hot buffers
FW = D * GTOK        # free elems per feature tile partition
RW = 8 * D           # rhs cols handed to PE (8 partial sums x 64 dims)


OHB = 8              # one-hot batch size (tiles per build instruction)


def tile_on_dve(j: int) -> bool:
    # ~45:17 DVE:Pool split: measured ~1.7us/tile on DVE (plus one-hot
    # batches + blend) vs ~4.2us/tile on Pool
    return j % 7 < 5


def build_nc(T: int, step_gt_warmup: bool):
    f32 = mybir.dt.float32
    bf16 = mybir.dt.bfloat16
    fp8 = mybir.dt.float8e4

    dcount = [0] * (T + 1)  # dcount[j+1] = #DVE tiles among 0..j
    pcount = [0] * (T + 1)
    for j in range(T):
        dcount[j + 1] = dcount[j] + (1 if tile_on_dve(j) else 0)
        pcount[j + 1] = pcount[j] + (0 if tile_on_dve(j) else 1)

    nc = bacc.Bacc("TRN2", target_bir_lowering=False, debug=False,
                   num_devices=N_CORES)

    feat = nc.dram_tensor("feature", [T * P, FW], bf16, kind="ExternalInput")
    meta = nc.dram_tensor("blk_meta", [P, T], f32, kind="ExternalInput")
    meta2 = nc.dram_tensor("blk_cnt", [P, T], bf16, kind="ExternalInput")
    proto = nc.dram_tensor("prototype", [P, D], f32, kind="ExternalInput")
    iota = nc.dram_tensor("iota", [P, P], f32, kind="ExternalInput")
    out = nc.dram_tensor("out", [P, D], f32, kind="ExternalOutput")

    ftiles = [nc.alloc_sbuf_tensor(f"ftile{b}", [P, FW], bf16) for b in range(NB)]
    scr_d = nc.alloc_sbuf_tensor("scr_d", [P, FW // 2], bf16)
    scr_p = nc.alloc_sbuf_tensor("scr_p", [P, FW // 2], bf16)
    iota_sb = nc.alloc_sbuf_tensor("iota_sb", [P, P], f32)
    meta_sb = nc.alloc_sbuf_tensor("meta_sb", [P, T], f32)
    meta2_sb = nc.alloc_sbuf_tensor("meta2_sb", [P, T], bf16)
    proto_sb = nc.alloc_sbuf_tensor("proto_sb", [P, D], f32)
    rhs = [nc.alloc_sbuf_tensor(f"rhs{i}", [P, RW], bf16) for i in range(NR)]
    oh = [nc.alloc_sbuf_tensor(f"oh{i}", [P, OHB * P], bf16) for i in range(2)]
    cnt = nc.alloc_sbuf_tensor("cnt", [P, 1], f32)
    rcp = nc.alloc_sbuf_tensor("rcp", [P, 1], f32)
    pres = nc.alloc_sbuf_tensor("pres", [P, 1], f32)
    znorm = nc.alloc_sbuf_tensor("znorm", [P, 1], f32)
    svec = nc.alloc_sbuf_tensor("svec", [P, 1], f32)
    means = nc.alloc_sbuf_tensor("means", [P, D], f32)
    dtile = nc.alloc_sbuf_tensor("dtile", [P, D], f32)
    otile = nc.alloc_sbuf_tensor("otile", [P, D], f32)

    psum_s = nc.alloc_psum_tensor("psum_s", [P, RW], f32)
    psum_c = nc.alloc_psum_tensor("psum_c", [P, 1], f32)
    sums = nc.alloc_sbuf_tensor("sums", [P, D], f32)

    lsems = [nc.alloc_semaphore(f"lsem{b}") for b in range(NB)]
    rsem_d = nc.alloc_semaphore("rsem_d")  # DVE reduces done
    rsem_p = nc.alloc_semaphore("rsem_p")  # Pool reduces done
    ohsem = nc.alloc_semaphore("ohsem")    # one-hots built
    msem = nc.alloc_semaphore("msem")      # matmuls done (1 per tile)
    psem = nc.alloc_semaphore("psem")      # preamble loads (3 x 16)
    bsem = nc.alloc_semaphore("bsem")      # blend done
    fsem = nc.alloc_semaphore("fsem")      # out store done
    vch = nc.alloc_semaphore("vch")        # blend chain

    def pstride(t):
        return t.ap().ap[0][0]

    def feat_tile_ap(j):
        return bass.AP(feat, j * P * FW, [[FW, P], [1, FW]])

    def ftile_red_ap(b):
        t = ftiles[b]
        return bass.AP(t, 0, [[pstride(t), P], [GTOK, E], [1, GTOK]])

    def col(t, c, w=1):
        return bass.AP(t, c, [[pstride(t), P], [1, w]])

    def wait_loaded_free(eng, j):
        """Wait until tile j's ftile is consumed (level 1 done)."""
        if tile_on_dve(j):
            eng.wait_ge(rsem_d, 2 * dcount[j + 1] - 1)
        else:
            eng.wait_ge(rsem_p, 2 * pcount[j + 1] - 1)

    def wait_reduced(eng, j):
        """Wait until tile j's rhs is ready (level 2 done)."""
        if tile_on_dve(j):
            eng.wait_ge(rsem_d, 2 * dcount[j + 1])
        else:
            eng.wait_ge(rsem_p, 2 * pcount[j + 1])

    def tree_reduce(eng, j, rsem, scr):
        """Two halving adds: ftile[b] fp8 [P, FW] -> scratch bf16 [P, FW/2]
        -> rhs[j%NR] bf16 [P, RW]. The ftile is free after level 1."""
        b = j % NB
        t = ftiles[b]
        eng.wait_ge(lsems[b], 16 * (j // NB + 1))
        h = FW // 2
        eng.tensor_tensor(
            bass.AP(scr, 0, [[pstride(scr), P], [1, h]]),
            bass.AP(t, 0, [[pstride(t), P], [1, h]]),
            bass.AP(t, h, [[pstride(t), P], [1, h]]),
            mybir.AluOpType.add,
        ).then_inc(rsem, 1)
        if j >= NR:
            eng.wait_ge(msem, j - NR + 1)  # rhs slot freed by matmul j-NR
        eng.tensor_tensor(
            rhs[j % NR].ap(),
            bass.AP(scr, 0, [[pstride(scr), P], [1, RW]]),
            bass.AP(scr, RW, [[pstride(scr), P], [1, RW]]),
            mybir.AluOpType.add,
        ).then_inc(rsem, 1)

    with nc.allow_low_precision("bf16 block sums; exact count col"), \
            nc.Block() as block:

        @block.scalar
        def _(scalar):
            scalar.dma_start(iota_sb.ap(), iota.ap()).then_inc(psem, 16)
            scalar.dma_start(meta_sb.ap(), meta.ap()).then_inc(psem, 16)
            scalar.dma_start(proto_sb.ap(), proto.ap()).then_inc(psem, 16)
            scalar.dma_start(meta2_sb.ap(), meta2.ap()).then_inc(psem, 16)

        @block.sync
        def _(sync):
            for j in range(T):
                b = j % NB
                if j >= NB:
                    wait_loaded_free(sync, j - NB)
                sync.dma_start(ftiles[b].ap(), feat_tile_ap(j)).then_inc(lsems[b], 16)
            sync.wait_ge(bsem, 1)
            sync.dma_start(out.ap(), otile.ap()).then_inc(fsem, 16)
            sync.wait_ge(fsem, 16)

        @block.gpsimd
        def _(gpsimd):
            for j in range(T):
                if not tile_on_dve(j):
                    tree_reduce(gpsimd, j, rsem_p, scr_p)

        @block.vector
        def _(vector):
            vector.wait_ge(psem, 64)
            for j in range(T):
                if j % OHB == 0:
                    b = j // OHB
                    nb = min(OHB, T - j)
                    if b >= 2:
                        vector.wait_ge(msem, OHB * (b - 1))
                    t = oh[b % 2]
                    vector.tensor_tensor(
                        bass.AP(t, 0, [[pstride(t), P], [P, nb], [1, P]]),
                        bass.AP(meta_sb, j, [[pstride(meta_sb), P], [1, nb], [0, P]]),
                        bass.AP(iota_sb, 0, [[pstride(iota_sb), P], [0, nb], [1, P]]),
                        mybir.AluOpType.is_equal,
                    ).then_inc(ohsem, 1)
                if tile_on_dve(j):
                    tree_reduce(vector, j, rsem_d, scr_d)

            # ---- blend ----
            vector.wait_ge(msem, T)
            vc = [0]

            def chain(ins):
                ins.then_inc(vch, 1)
                vc[0] += 1
                vector.wait_ge(vch, vc[0])

            chain(vector.tensor_reduce(
                sums.ap(),
                bass.AP(psum_s, 0, [[pstride(psum_s), P], [1, D], [D, 8]]),
                axis=mybir.AxisListType.X, op=mybir.AluOpType.add,
            ))
            chain(vector.tensor_copy(cnt.ap(), psum_c.ap()))
            chain(vector.tensor_scalar_max(rcp.ap(), cnt.ap(), 1.0))
            chain(vector.reciprocal(rcp.ap(), rcp.ap()))
            chain(vector.tensor_scalar(pres.ap(), cnt.ap(), 0.5, None,
                                       mybir.AluOpType.is_gt))
            if step_gt_warmup:
                chain(vector.tensor_reduce(
                    znorm.ap(), proto_sb.ap(),
                    axis=mybir.AxisListType.X, op=mybir.AluOpType.max,
                    apply_absolute_value=True,
                ))
                chain(vector.tensor_scalar(svec.ap(), znorm.ap(), 0.0, None,
                                           mybir.AluOpType.is_equal))
            else:
                chain(vector.memset(svec.ap(), 1.0))
            # svec = pres * (0.1 + 0.9*use_new)
            chain(vector.tensor_scalar(svec.ap(), svec.ap(), LAM, 1.0 - LAM,
                                       mybir.AluOpType.mult,
                                       mybir.AluOpType.add))
            chain(vector.tensor_tensor(svec.ap(), svec.ap(), pres.ap(),
                                       mybir.AluOpType.mult))
            chain(vector.tensor_scalar_mul(means.ap(), sums.ap(), col(rcp, 0)))
            chain(vector.tensor_tensor(dtile.ap(), means.ap(), proto_sb.ap(),
                                       mybir.AluOpType.subtract))
            vector.scalar_tensor_tensor(
                otile.ap(), dtile.ap(), col(svec, 0), proto_sb.ap(),
                mybir.AluOpType.mult, mybir.AluOpType.add,
            ).then_inc(bsem, 1)

        @block.tensor
        def _(tensor):
            tensor.wait_ge(psem, 64)
            for j in range(T):
                wait_reduced(tensor, j)
                tensor.wait_ge(ohsem, j // OHB + 1)
                oh_t = oh[(j // OHB) % 2]
                oh_ap = bass.AP(oh_t, (j % OHB) * P, [[pstride(oh_t), P], [1, P]])
                tensor.matmul(
                    psum_s.ap(), oh_ap, rhs[j % NR].ap(),
                    start=(j == 0), stop=(j == T - 1),
                )
                tensor.matmul(
                    psum_c.ap(), oh_ap,
                    bass.AP(meta2_sb, j, [[pstride(meta2_sb), P], [1, 1]]),
                    start=(j == 0), stop=(j == T - 1),
                ).then_inc(msem, 1)

    nc.compile()
    return nc


def shard_inputs(feature, label, prototype):
    """Returns (in_maps, cls_lists, T)."""
    import ml_dtypes
    bf16 = ml_dtypes.bfloat16
    fp8 = ml_dtypes.float8_e4m3

    counts = np.bincount(label, minlength=C)

    # greedy: biggest class -> least-loaded core (cap 128 classes/core)
    order_cls = np.argsort(-counts, kind="stable")
    core_load = np.zeros(N_CORES, dtype=np.int64)
    core_ncls = np.zeros(N_CORES, dtype=np.int64)
    cls_lists = [[] for _ in range(N_CORES)]
    nblk = (counts + GTOK - 1) // GTOK  # blocks per class
    for c in order_cls:
        k = min((k for k in range(N_CORES) if core_ncls[k] < P),
                key=lambda k: core_load[k])
        cls_lists[k].append(c)
        core_load[k] += nblk[c]
        core_ncls[k] += 1

    T = int(max(1, -(-core_load.max() // P)))
    cap_blk = T * P
    cap_tok = cap_blk * GTOK

    sort_order = np.argsort(label, kind="stable")
    starts = np.zeros(C + 1, dtype=np.int64)
    np.cumsum(counts, out=starts[1:])

    feat_bf = np.ascontiguousarray(feature, dtype=np.float32).astype(bf16)

    src_all = np.full(N_CORES * cap_tok, -1, dtype=np.int64)
    metas = []
    for k in range(N_CORES):
        base = k * cap_tok
        pos = 0
        mcls = np.zeros(cap_blk, dtype=np.float32)
        mcnt = np.zeros(cap_blk, dtype=np.float32)
        blk = 0
        for li, c in enumerate(cls_lists[k]):
            ncv = int(counts[c])
            if ncv:
                src_all[base + pos: base + pos + ncv] = \
                    sort_order[starts[c]: starts[c] + ncv]
            nb = int(nblk[c])
            if nb:
                mcls[blk: blk + nb] = li
                mcnt[blk: blk + nb] = GTOK
                mcnt[blk + nb - 1] = ncv - (nb - 1) * GTOK
            pos += nb * GTOK
            blk += nb
        metas.append((mcls, mcnt))

    nblk_tot = N_CORES * cap_blk
    valid = src_all >= 0
    tok = np.zeros((nblk_tot * GTOK, D), dtype=bf16)
    tok[valid] = feat_bf[src_all[valid]]
    arr = tok.reshape(N_CORES, T * P, FW)

    proto32 = np.ascontiguousarray(prototype, dtype=np.float32)
    iota_arr = np.tile(np.arange(P, dtype=np.float32), (P, 1))
    in_maps = []
    for k in range(N_CORES):
        cl = np.asarray(cls_lists[k], dtype=np.int64)
        pk = np.zeros((P, D), dtype=np.float32)
        pk[: len(cl)] = proto32[cl]
        # block b=(tile j, partition p) -> meta[p, j]
        meta_k = np.ascontiguousarray(metas[k][0].reshape(T, P).T)
        meta2_k = np.ascontiguousarray(
            metas[k][1].reshape(T, P).T.astype(bf16))
        in_maps.append({
            "feature": np.ascontiguousarray(arr[k]),
            "blk_meta": meta_k,
            "blk_cnt": meta2_k,
            "prototype": pk,
            "iota": iota_arr,
        })
    return in_maps, cls_lists, T


_NC_CACHE = {}


def run(inputs: dict, trace: bool = False):
    feature = np.asarray(inputs["feature"])
    label = np.asarray(inputs["label"], dtype=np.int64)
    prototype = np.asarray(inputs["prototype"])
    step = int(np.asarray(inputs["step"]))

    in_maps, cls_lists, T = shard_inputs(feature, label, prototype)
    key = (T, step > WARMUP_STEP)
    if key not in _NC_CACHE:
        _NC_CACHE[key] = build_nc(T, step > WARMUP_STEP)
    nc = _NC_CACHE[key]
    res = bass_utils.run_bass_kernel_spmd(
        nc, in_maps, core_ids=list(range(N_CORES)), trace=trace,
    )
    out = np.ascontiguousarray(prototype, dtype=np.float32).copy()
    for k in range(N_CORES):
        cl = np.asarray(cls_lists[k], dtype=np.int64)
        ok = np.asarray(res.results[k]["out"], dtype=np.float32)
        out[cl] = ok[: len(cl)]
    return out, res


def kernel(**inputs) -> np.ndarray:
    out, _ = run(inputs, trace=False)
    return out


# revision 17
# speedup vs baseline: 1.0348x; 1.0348x over previous
"""Trainium2 Bass kernel for the prototype-bank scatter-mean EMA update
(nn_Bank): class-sharded sorted-segment reduction across 8 NeuronCores.

Host (index/layout work only; all FP reduction arithmetic is on device):
  * argsort labels; assign each class to one core (greedy token balance,
    <=128 classes/core); concatenate each core's class segments, zero-
    padding every class to a multiple of GTOK tokens so each GTOK-token
    "block" is single-class.
  * feature blocks are packed feature-major [65, GTOK] in bf16: rows
    0..63 = the block's GTOK token features (transposed), row 64 = a
    1.0/0.0 valid-token indicator (so the same reduction that produces
    block feature sums also produces block counts).
  * per-core metadata: block -> local-class id; prototype rows for the
    core's classes; an iota row table.

Device, per core (T tiles; tile = 128 blocks = 128*GTOK tokens):
  1. Stream feature tiles [128, 65*GTOK] bf16 HBM->SBUF (sync queue).
  2. Block sums: tiles alternate between DVE tensor_reduce (true sums)
     and Pool avg-pool (sums/GTOK) -> rhs [128, 65] bf16.
  3. DVE builds one-hot lhsT [128 blocks, 128 local classes] bf16 via
     (iota == cls) * scale, scale = GTOK for Pool tiles (undoes the avg)
     and 1 for DVE tiles.
  4. PE: psum[cls, 65] += oh^T @ rhs, PSUM-accumulated over all T tiles
     -> per-class feature sums (cols 0..63) and counts (col 64).
  5. Blend: means = sums/max(cnt,1); out = proto + s*(means-proto) with
     s = present * (0.1 + 0.9*use_new)  [step>warmup branch].
  6. DMA out [128, D]; host scatters per-core rows back to [1000, 64].

No collective: every class is fully owned by one core.
"""

import numpy as np

import concourse.bacc as bacc
import concourse.bass as bass
import concourse.mybir as mybir
from concourse import bass_utils

C = 1000
D = 64
E = D + 1            # feature dims + count indicator (metadata only)
P = 128
GTOK = 32            # tokens per block (class padding granularity)
LAM = 0.9
WARMUP_STEP = 1000
N_CORES = 8
NB = 12              # feature tile buffers
NR = 6               # rhs buffers
NO = 6               # one-hot tile buffers
FW = D * GTOK        # free elems per feature tile partition
RW = 8 * D           # rhs cols handed to PE (8 partial sums x 64 dims)


OHB = 8              # one-hot batch size (tiles per build instruction)


def tile_on_dve(j: int) -> bool:
    # ~50:12 DVE:Pool split: single level-1 add ~1.15us/tile on DVE vs
    # ~4.5us/tile on Pool
    return j % 5 != 4


def build_nc(T: int, step_gt_warmup: bool):
    f32 = mybir.dt.float32
    bf16 = mybir.dt.bfloat16
    fp8 = mybir.dt.float8e4

    dcount = [0] * (T + 1)  # dcount[j+1] = #DVE tiles among 0..j
    pcount = [0] * (T + 1)
    for j in range(T):
        dcount[j + 1] = dcount[j] + (1 if tile_on_dve(j) else 0)
        pcount[j + 1] = pcount[j] + (0 if tile_on_dve(j) else 1)

    nc = bacc.Bacc("TRN2", target_bir_lowering=False, debug=False,
                   num_devices=N_CORES)

    feat = nc.dram_tensor("feature", [T * P, FW], bf16, kind="ExternalInput")
    oh_rows = nc.dram_tensor("oh_rows", [T * P, P], bf16, kind="ExternalInput")
    meta2 = nc.dram_tensor("blk_cnt", [P, T], bf16, kind="ExternalInput")
    proto = nc.dram_tensor("prototype", [P, D], f32, kind="ExternalInput")
    out = nc.dram_tensor("out", [P, D], f32, kind="ExternalOutput")

    ftiles = [nc.alloc_sbuf_tensor(f"ftile{b}", [P, FW], bf16) for b in range(NB)]
    meta2_sb = nc.alloc_sbuf_tensor("meta2_sb", [P, T], bf16)
    proto_sb = nc.alloc_sbuf_tensor("proto_sb", [P, D], f32)
    rhs = [nc.alloc_sbuf_tensor(f"rhs{i}", [P, FW // 2], bf16) for i in range(NR)]
    oh = [nc.alloc_sbuf_tensor(f"oh{i}", [P, P], bf16) for i in range(NO)]
    cnt = nc.alloc_sbuf_tensor("cnt", [P, 1], f32)
    rcp = nc.alloc_sbuf_tensor("rcp", [P, 1], f32)
    pres = nc.alloc_sbuf_tensor("pres", [P, 1], f32)
    znorm = nc.alloc_sbuf_tensor("znorm", [P, 1], f32)
    svec = nc.alloc_sbuf_tensor("svec", [P, 1], f32)
    means = nc.alloc_sbuf_tensor("means", [P, D], f32)
    dtile = nc.alloc_sbuf_tensor("dtile", [P, D], f32)
    otile = nc.alloc_sbuf_tensor("otile", [P, D], f32)

    psum_a = nc.alloc_psum_tensor("psum_a", [P, RW], f32)
    psum_b = nc.alloc_psum_tensor("psum_b", [P, RW], f32)
    psum_c = nc.alloc_psum_tensor("psum_c", [P, 1], f32)
    sums = nc.alloc_sbuf_tensor("sums", [P, D], f32)

    lsems = [nc.alloc_semaphore(f"lsem{b}") for b in range(NB)]
    rsem_d = nc.alloc_semaphore("rsem_d")  # DVE reduces done
    rsem_p = nc.alloc_semaphore("rsem_p")  # Pool reduces done
    osems = [nc.alloc_semaphore(f"osem{i}") for i in range(NO)]
    msem = nc.alloc_semaphore("msem")      # matmuls done (1 per tile)
    psem = nc.alloc_semaphore("psem")      # preamble loads (3 x 16)
    bsem = nc.alloc_semaphore("bsem")      # blend done
    fsem = nc.alloc_semaphore("fsem")      # out store done
    vch = nc.alloc_semaphore("vch")        # blend chain

    def pstride(t):
        return t.ap().ap[0][0]

    def feat_tile_ap(j):
        return bass.AP(feat, j * P * FW, [[FW, P], [1, FW]])

    def ftile_red_ap(b):
        t = ftiles[b]
        return bass.AP(t, 0, [[pstride(t), P], [GTOK, E], [1, GTOK]])

    def col(t, c, w=1):
        return bass.AP(t, c, [[pstride(t), P], [1, w]])

    def wait_reduced(eng, j):
        """Wait until tile j's rhs is ready."""
        if tile_on_dve(j):
            eng.wait_ge(rsem_d, dcount[j + 1])
        else:
            eng.wait_ge(rsem_p, pcount[j + 1])

    def tree_reduce(eng, j, rsem):
        """One halving add: ftile[b] [P, FW] -> rhs[j%NR] [P, FW/2]."""
        b = j % NB
        t = ftiles[b]
        eng.wait_ge(lsems[b], 16 * (j // NB + 1))
        if j >= NR:
            eng.wait_ge(msem, j - NR + 1)  # rhs slot freed by matmuls j-NR
        h = FW // 2
        eng.tensor_tensor(
            rhs[j % NR].ap(),
            bass.AP(t, 0, [[pstride(t), P], [1, h]]),
            bass.AP(t, h, [[pstride(t), P], [1, h]]),
            mybir.AluOpType.add,
        ).then_inc(rsem, 1)

    with nc.allow_low_precision("bf16 block sums; exact count col"), \
            nc.Block() as block:

        @block.scalar
        def _(scalar):
            scalar.dma_start(proto_sb.ap(), proto.ap()).then_inc(psem, 16)
            scalar.dma_start(meta2_sb.ap(), meta2.ap()).then_inc(psem, 16)
            for j in range(T):
                if j >= NO:
                    scalar.wait_ge(msem, j - NO + 1)
                scalar.dma_start(
                    oh[j % NO].ap(),
                    bass.AP(oh_rows, j * P * P, [[P, P], [1, P]]),
                ).then_inc(osems[j % NO], 16)

        @block.sync
        def _(sync):
            for j in range(T):
                b = j % NB
                if j >= NB:
                    wait_reduced(sync, j - NB)
                sync.dma_start(ftiles[b].ap(), feat_tile_ap(j)).then_inc(lsems[b], 16)
            sync.wait_ge(bsem, 1)
            sync.dma_start(out.ap(), otile.ap()).then_inc(fsem, 16)
            sync.wait_ge(fsem, 16)

        @block.gpsimd
        def _(gpsimd):
            for j in range(T):
                if not tile_on_dve(j):
                    tree_reduce(gpsimd, j, rsem_p)

        @block.vector
        def _(vector):
            vector.wait_ge(psem, 32)
            for j in range(T):
                if tile_on_dve(j):
                    tree_reduce(vector, j, rsem_d)

            # ---- blend ----
            vector.wait_ge(msem, T)
            vc = [0]

            def chain(ins):
                ins.then_inc(vch, 1)
                vc[0] += 1
                vector.wait_ge(vch, vc[0])

            chain(vector.tensor_reduce(
                sums.ap(),
                bass.AP(psum_a, 0, [[pstride(psum_a), P], [1, D], [D, 8]]),
                axis=mybir.AxisListType.X, op=mybir.AluOpType.add,
            ))
            chain(vector.tensor_reduce(
                dtile.ap(),
                bass.AP(psum_b, 0, [[pstride(psum_b), P], [1, D], [D, 8]]),
                axis=mybir.AxisListType.X, op=mybir.AluOpType.add,
            ))
            chain(vector.tensor_tensor(sums.ap(), sums.ap(), dtile.ap(),
                                       mybir.AluOpType.add))
            chain(vector.tensor_copy(cnt.ap(), psum_c.ap()))
            chain(vector.tensor_scalar_max(rcp.ap(), cnt.ap(), 1.0))
            chain(vector.reciprocal(rcp.ap(), rcp.ap()))
            chain(vector.tensor_scalar(pres.ap(), cnt.ap(), 0.5, None,
                                       mybir.AluOpType.is_gt))
            if step_gt_warmup:
                chain(vector.tensor_reduce(
                    znorm.ap(), proto_sb.ap(),
                    axis=mybir.AxisListType.X, op=mybir.AluOpType.max,
                    apply_absolute_value=True,
                ))
                chain(vector.tensor_scalar(svec.ap(), znorm.ap(), 0.0, None,
                                           mybir.AluOpType.is_equal))
            else:
                chain(vector.memset(svec.ap(), 1.0))
            # svec = pres * (0.1 + 0.9*use_new)
            chain(vector.tensor_scalar(svec.ap(), svec.ap(), LAM, 1.0 - LAM,
                                       mybir.AluOpType.mult,
                                       mybir.AluOpType.add))
            chain(vector.tensor_tensor(svec.ap(), svec.ap(), pres.ap(),
                                       mybir.AluOpType.mult))
            chain(vector.tensor_scalar_mul(means.ap(), sums.ap(), col(rcp, 0)))
            chain(vector.tensor_tensor(dtile.ap(), means.ap(), proto_sb.ap(),
                                       mybir.AluOpType.subtract))
            vector.scalar_tensor_tensor(
                otile.ap(), dtile.ap(), col(svec, 0), proto_sb.ap(),
                mybir.AluOpType.mult, mybir.AluOpType.add,
            ).then_inc(bsem, 1)

        @block.tensor
        def _(tensor):
            tensor.wait_ge(psem, 32)
            for j in range(T):
                wait_reduced(tensor, j)
                tensor.wait_ge(osems[j % NO], 16 * (j // NO + 1))
                oh_ap = oh[j % NO].ap()
                r = rhs[j % NR]
                tensor.matmul(
                    psum_a.ap(), oh_ap,
                    bass.AP(r, 0, [[pstride(r), P], [1, RW]]),
                    start=(j == 0), stop=(j == T - 1),
                )
                tensor.matmul(
                    psum_b.ap(), oh_ap,
                    bass.AP(r, RW, [[pstride(r), P], [1, RW]]),
                    start=(j == 0), stop=(j == T - 1),
                )
                tensor.matmul(
                    psum_c.ap(), oh_ap,
                    bass.AP(meta2_sb, j, [[pstride(meta2_sb), P], [1, 1]]),
                    start=(j == 0), stop=(j == T - 1),
                ).then_inc(msem, 1)

    nc.compile()
    return nc


def shard_inputs(feature, label, prototype):
    """Returns (in_maps, cls_lists, T)."""
    import ml_dtypes
    bf16 = ml_dtypes.bfloat16
    fp8 = ml_dtypes.float8_e4m3

    counts = np.bincount(label, minlength=C)

    # greedy: biggest class -> least-loaded core (cap 128 classes/core)
    order_cls = np.argsort(-counts, kind="stable")
    core_load = np.zeros(N_CORES, dtype=np.int64)
    core_ncls = np.zeros(N_CORES, dtype=np.int64)
    cls_lists = [[] for _ in range(N_CORES)]
    nblk = (counts + GTOK - 1) // GTOK  # blocks per class
    for c in order_cls:
        k = min((k for k in range(N_CORES) if core_ncls[k] < P),
                key=lambda k: core_load[k])
        cls_lists[k].append(c)
        core_load[k] += nblk[c]
        core_ncls[k] += 1

    T = int(max(1, -(-core_load.max() // P)))
    cap_blk = T * P
    cap_tok = cap_blk * GTOK

    sort_order = np.argsort(label, kind="stable")
    starts = np.zeros(C + 1, dtype=np.int64)
    np.cumsum(counts, out=starts[1:])

    feat_bf = np.ascontiguousarray(feature, dtype=np.float32).astype(bf16)

    src_all = np.full(N_CORES * cap_tok, -1, dtype=np.int64)
    metas = []
    for k in range(N_CORES):
        base = k * cap_tok
        pos = 0
        mcls = np.full(cap_blk, -1, dtype=np.int64)
        mcnt = np.zeros(cap_blk, dtype=np.float32)
        blk = 0
        for li, c in enumerate(cls_lists[k]):
            ncv = int(counts[c])
            if ncv:
                src_all[base + pos: base + pos + ncv] = \
                    sort_order[starts[c]: starts[c] + ncv]
            nb = int(nblk[c])
            if nb:
                mcls[blk: blk + nb] = li
                mcnt[blk: blk + nb] = GTOK
                mcnt[blk + nb - 1] = ncv - (nb - 1) * GTOK
            pos += nb * GTOK
            blk += nb
        metas.append((mcls, mcnt))

    nblk_tot = N_CORES * cap_blk
    valid = src_all >= 0
    tok = np.zeros((nblk_tot * GTOK, D), dtype=bf16)
    tok[valid] = feat_bf[src_all[valid]]
    arr = tok.reshape(N_CORES, T * P, FW)

    proto32 = np.ascontiguousarray(prototype, dtype=np.float32)
    in_maps = []
    for k in range(N_CORES):
        cl = np.asarray(cls_lists[k], dtype=np.int64)
        pk = np.zeros((P, D), dtype=np.float32)
        pk[: len(cl)] = proto32[cl]
        # one-hot row per block (zero row for pad blocks)
        mcls = metas[k][0]
        ohh = np.zeros((cap_blk, P), dtype=bf16)
        real = mcls >= 0
        ohh[np.flatnonzero(real), mcls[real]] = 1
        # block b=(tile j, partition p) -> blk_cnt[p, j]
        meta2_k = np.ascontiguousarray(
            metas[k][1].reshape(T, P).T.astype(bf16))
        in_maps.append({
            "feature": np.ascontiguousarray(arr[k]),
            "oh_rows": ohh,
            "blk_cnt": meta2_k,
            "prototype": pk,
        })
    return in_maps, cls_lists, T


_NC_CACHE = {}


def run(inputs: dict, trace: bool = False):
    feature = np.asarray(inputs["feature"])
    label = np.asarray(inputs["label"], dtype=np.int64)
    prototype = np.asarray(inputs["prototype"])
    step = int(np.asarray(inputs["step"]))

    in_maps, cls_lists, T = shard_inputs(feature, label, prototype)
    key = (T, step > WARMUP_STEP)
    if key not in _NC_CACHE:
        _NC_CACHE[key] = build_nc(T, step > WARMUP_STEP)
    nc = _NC_CACHE[key]
    res = bass_utils.run_bass_kernel_spmd(
        nc, in_maps, core_ids=list(range(N_CORES)), trace=trace,
    )
    out = np.ascontiguousarray(prototype, dtype=np.float32).copy()
    for k in range(N_CORES):
        cl = np.asarray(cls_lists[k], dtype=np.int64)
        ok = np.asarray(res.results[k]["out"], dtype=np.float32)
        out[cl] = ok[: len(cl)]
    return out, res


def kernel(**inputs) -> np.ndarray:
    out, _ = run(inputs, trace=False)
    return out


# revision 18
# speedup vs baseline: 1.1699x; 1.1306x over previous
"""Trainium2 Bass kernel for the prototype-bank scatter-mean EMA update
(nn_Bank): class-sharded sorted-segment reduction across 8 NeuronCores.

Host (index/layout work only; all FP reduction arithmetic is on device):
  * argsort labels; assign each class to one core (greedy token balance,
    <=128 classes/core); concatenate each core's class segments, zero-
    padding every class to a multiple of GTOK tokens so each GTOK-token
    "block" is single-class.
  * feature blocks are packed feature-major [65, GTOK] in bf16: rows
    0..63 = the block's GTOK token features (transposed), row 64 = a
    1.0/0.0 valid-token indicator (so the same reduction that produces
    block feature sums also produces block counts).
  * per-core metadata: block -> local-class id; prototype rows for the
    core's classes; an iota row table.

Device, per core (T tiles; tile = 128 blocks = 128*GTOK tokens):
  1. Stream feature tiles [128, 65*GTOK] bf16 HBM->SBUF (sync queue).
  2. Block sums: tiles alternate between DVE tensor_reduce (true sums)
     and Pool avg-pool (sums/GTOK) -> rhs [128, 65] bf16.
  3. DVE builds one-hot lhsT [128 blocks, 128 local classes] bf16 via
     (iota == cls) * scale, scale = GTOK for Pool tiles (undoes the avg)
     and 1 for DVE tiles.
  4. PE: psum[cls, 65] += oh^T @ rhs, PSUM-accumulated over all T tiles
     -> per-class feature sums (cols 0..63) and counts (col 64).
  5. Blend: means = sums/max(cnt,1); out = proto + s*(means-proto) with
     s = present * (0.1 + 0.9*use_new)  [step>warmup branch].
  6. DMA out [128, D]; host scatters per-core rows back to [1000, 64].

No collective: every class is fully owned by one core.
"""

import numpy as np

import concourse.bacc as bacc
import concourse.bass as bass
import concourse.mybir as mybir
from concourse import bass_utils

C = 1000
D = 64
E = D + 1            # feature dims + count indicator (metadata only)
P = 128
GTOK = 32            # tokens per block (class padding granularity)
LAM = 0.9
WARMUP_STEP = 1000
N_CORES = 8
NB = 12              # feature tile buffers
NR = 6               # rhs buffers
NO = 6               # one-hot tile buffers
FW = D * GTOK        # free elems per feature tile partition
RW = 8 * D           # rhs cols handed to PE (8 partial sums x 64 dims)


OHB = 8              # one-hot batch size (tiles per build instruction)


def tile_on_dve(j: int) -> bool:
    # hybrid: DVE tiles get a 2-level tree (PE streams 512 cols), Pool
    # tiles 1 level (PE streams 2x512); ~44:18 balances DVE/Pool/PE
    return j % 7 < 5


def build_nc(T: int, step_gt_warmup: bool):
    f32 = mybir.dt.float32
    bf16 = mybir.dt.bfloat16
    fp8 = mybir.dt.float8e4

    dcount = [0] * (T + 1)  # dcount[j+1] = #DVE tiles among 0..j
    pcount = [0] * (T + 1)
    for j in range(T):
        dcount[j + 1] = dcount[j] + (1 if tile_on_dve(j) else 0)
        pcount[j + 1] = pcount[j] + (0 if tile_on_dve(j) else 1)

    nc = bacc.Bacc("TRN2", target_bir_lowering=False, debug=False,
                   num_devices=N_CORES)

    feat = nc.dram_tensor("feature", [T * P, FW], bf16, kind="ExternalInput")
    oh_rows = nc.dram_tensor("oh_rows", [T * P, P], bf16, kind="ExternalInput")
    meta2 = nc.dram_tensor("blk_cnt", [P, T], bf16, kind="ExternalInput")
    proto = nc.dram_tensor("prototype", [P, D], f32, kind="ExternalInput")
    out = nc.dram_tensor("out", [P, D], f32, kind="ExternalOutput")

    ftiles = [nc.alloc_sbuf_tensor(f"ftile{b}", [P, FW], bf16) for b in range(NB)]
    meta2_sb = nc.alloc_sbuf_tensor("meta2_sb", [P, T], bf16)
    proto_sb = nc.alloc_sbuf_tensor("proto_sb", [P, D], f32)
    rhs = [nc.alloc_sbuf_tensor(f"rhs{i}", [P, FW // 2], bf16) for i in range(NR)]
    oh = [nc.alloc_sbuf_tensor(f"oh{i}", [P, P], bf16) for i in range(NO)]
    cnt = nc.alloc_sbuf_tensor("cnt", [P, 1], f32)
    rcp = nc.alloc_sbuf_tensor("rcp", [P, 1], f32)
    pres = nc.alloc_sbuf_tensor("pres", [P, 1], f32)
    znorm = nc.alloc_sbuf_tensor("znorm", [P, 1], f32)
    svec = nc.alloc_sbuf_tensor("svec", [P, 1], f32)
    means = nc.alloc_sbuf_tensor("means", [P, D], f32)
    dtile = nc.alloc_sbuf_tensor("dtile", [P, D], f32)
    otile = nc.alloc_sbuf_tensor("otile", [P, D], f32)

    psum_a = nc.alloc_psum_tensor("psum_a", [P, RW], f32)
    psum_b = nc.alloc_psum_tensor("psum_b", [P, RW], f32)
    psum_c = nc.alloc_psum_tensor("psum_c", [P, 1], f32)
    sums = nc.alloc_sbuf_tensor("sums", [P, D], f32)

    lsems = [nc.alloc_semaphore(f"lsem{b}") for b in range(NB)]
    rsem_d = nc.alloc_semaphore("rsem_d")  # DVE reduces done
    rsem_p = nc.alloc_semaphore("rsem_p")  # Pool reduces done
    osems = [nc.alloc_semaphore(f"osem{i}") for i in range(NO)]
    msem = nc.alloc_semaphore("msem")      # matmuls done (1 per tile)
    psem = nc.alloc_semaphore("psem")      # preamble loads (3 x 16)
    bsem = nc.alloc_semaphore("bsem")      # blend done
    fsem = nc.alloc_semaphore("fsem")      # out store done
    vch = nc.alloc_semaphore("vch")        # blend chain

    def pstride(t):
        return t.ap().ap[0][0]

    def feat_tile_ap(j):
        return bass.AP(feat, j * P * FW, [[FW, P], [1, FW]])

    def ftile_red_ap(b):
        t = ftiles[b]
        return bass.AP(t, 0, [[pstride(t), P], [GTOK, E], [1, GTOK]])

    def col(t, c, w=1):
        return bass.AP(t, c, [[pstride(t), P], [1, w]])

    def wait_reduced(eng, j):
        """Wait until tile j's rhs is ready."""
        if tile_on_dve(j):
            eng.wait_ge(rsem_d, dcount[j + 1])
        else:
            eng.wait_ge(rsem_p, pcount[j + 1])

    def tree_reduce(eng, j, rsem, levels):
        """1 or 2 halving adds: ftile[b] [P, FW] -> rhs[j%NR] ([P, FW/2]
        in cols 0:FW/2, or [P, RW] in cols 0:RW for levels=2)."""
        b = j % NB
        t = ftiles[b]
        r = rhs[j % NR]
        eng.wait_ge(lsems[b], 16 * (j // NB + 1))
        h = FW // 2
        if levels == 2:
            eng.tensor_tensor(
                bass.AP(t, 0, [[pstride(t), P], [1, h]]),
                bass.AP(t, 0, [[pstride(t), P], [1, h]]),
                bass.AP(t, h, [[pstride(t), P], [1, h]]),
                mybir.AluOpType.add,
            )
            if j >= NR:
                eng.wait_ge(msem, j - NR + 1)
            eng.tensor_tensor(
                bass.AP(r, 0, [[pstride(r), P], [1, RW]]),
                bass.AP(t, 0, [[pstride(t), P], [1, RW]]),
                bass.AP(t, RW, [[pstride(t), P], [1, RW]]),
                mybir.AluOpType.add,
            ).then_inc(rsem, 1)
        else:
            if j >= NR:
                eng.wait_ge(msem, j - NR + 1)
            eng.tensor_tensor(
                bass.AP(r, 0, [[pstride(r), P], [1, h]]),
                bass.AP(t, 0, [[pstride(t), P], [1, h]]),
                bass.AP(t, h, [[pstride(t), P], [1, h]]),
                mybir.AluOpType.add,
            ).then_inc(rsem, 1)

    with nc.allow_low_precision("bf16 block sums; exact count col"), \
            nc.Block() as block:

        @block.scalar
        def _(scalar):
            scalar.dma_start(proto_sb.ap(), proto.ap()).then_inc(psem, 16)
            scalar.dma_start(meta2_sb.ap(), meta2.ap()).then_inc(psem, 16)
            for j in range(T):
                if j >= NO:
                    scalar.wait_ge(msem, j - NO + 1)
                scalar.dma_start(
                    oh[j % NO].ap(),
                    bass.AP(oh_rows, j * P * P, [[P, P], [1, P]]),
                ).then_inc(osems[j % NO], 16)

        @block.sync
        def _(sync):
            for j in range(T):
                b = j % NB
                if j >= NB:
                    wait_reduced(sync, j - NB)
                sync.dma_start(ftiles[b].ap(), feat_tile_ap(j)).then_inc(lsems[b], 16)
            sync.wait_ge(bsem, 1)
            sync.dma_start(out.ap(), otile.ap()).then_inc(fsem, 16)
            sync.wait_ge(fsem, 16)

        @block.gpsimd
        def _(gpsimd):
            for j in range(T):
                if not tile_on_dve(j):
                    tree_reduce(gpsimd, j, rsem_p, 1)

        @block.vector
        def _(vector):
            vector.wait_ge(psem, 32)
            for j in range(T):
                if tile_on_dve(j):
                    tree_reduce(vector, j, rsem_d, 2)

            # ---- blend ----
            vector.wait_ge(msem, T)
            vc = [0]

            def chain(ins):
                ins.then_inc(vch, 1)
                vc[0] += 1
                vector.wait_ge(vch, vc[0])

            chain(vector.tensor_reduce(
                sums.ap(),
                bass.AP(psum_a, 0, [[pstride(psum_a), P], [1, D], [D, 8]]),
                axis=mybir.AxisListType.X, op=mybir.AluOpType.add,
            ))
            chain(vector.tensor_reduce(
                dtile.ap(),
                bass.AP(psum_b, 0, [[pstride(psum_b), P], [1, D], [D, 8]]),
                axis=mybir.AxisListType.X, op=mybir.AluOpType.add,
            ))
            chain(vector.tensor_tensor(sums.ap(), sums.ap(), dtile.ap(),
                                       mybir.AluOpType.add))
            chain(vector.tensor_copy(cnt.ap(), psum_c.ap()))
            chain(vector.tensor_scalar_max(rcp.ap(), cnt.ap(), 1.0))
            chain(vector.reciprocal(rcp.ap(), rcp.ap()))
            chain(vector.tensor_scalar(pres.ap(), cnt.ap(), 0.5, None,
                                       mybir.AluOpType.is_gt))
            if step_gt_warmup:
                chain(vector.tensor_reduce(
                    znorm.ap(), proto_sb.ap(),
                    axis=mybir.AxisListType.X, op=mybir.AluOpType.max,
                    apply_absolute_value=True,
                ))
                chain(vector.tensor_scalar(svec.ap(), znorm.ap(), 0.0, None,
                                           mybir.AluOpType.is_equal))
            else:
                chain(vector.memset(svec.ap(), 1.0))
            # svec = pres * (0.1 + 0.9*use_new)
            chain(vector.tensor_scalar(svec.ap(), svec.ap(), LAM, 1.0 - LAM,
                                       mybir.AluOpType.mult,
                                       mybir.AluOpType.add))
            chain(vector.tensor_tensor(svec.ap(), svec.ap(), pres.ap(),
                                       mybir.AluOpType.mult))
            chain(vector.tensor_scalar_mul(means.ap(), sums.ap(), col(rcp, 0)))
            chain(vector.tensor_tensor(dtile.ap(), means.ap(), proto_sb.ap(),
                                       mybir.AluOpType.subtract))
            vector.scalar_tensor_tensor(
                otile.ap(), dtile.ap(), col(svec, 0), proto_sb.ap(),
                mybir.AluOpType.mult, mybir.AluOpType.add,
            ).then_inc(bsem, 1)

        @block.tensor
        def _(tensor):
            pool_tiles = [j for j in range(T) if not tile_on_dve(j)]
            tensor.wait_ge(psem, 32)
            for j in range(T):
                wait_reduced(tensor, j)
                tensor.wait_ge(osems[j % NO], 16 * (j // NO + 1))
                oh_ap = oh[j % NO].ap()
                r = rhs[j % NR]
                tensor.matmul(
                    psum_a.ap(), oh_ap,
                    bass.AP(r, 0, [[pstride(r), P], [1, RW]]),
                    start=(j == 0), stop=(j == T - 1),
                )
                if not tile_on_dve(j):
                    tensor.matmul(
                        psum_b.ap(), oh_ap,
                        bass.AP(r, RW, [[pstride(r), P], [1, RW]]),
                        start=(j == pool_tiles[0]), stop=(j == pool_tiles[-1]),
                    )
                tensor.matmul(
                    psum_c.ap(), oh_ap,
                    bass.AP(meta2_sb, j, [[pstride(meta2_sb), P], [1, 1]]),
                    start=(j == 0), stop=(j == T - 1),
                ).then_inc(msem, 1)

    nc.compile()
    return nc


def shard_inputs(feature, label, prototype):
    """Returns (in_maps, cls_lists, T)."""
    import ml_dtypes
    bf16 = ml_dtypes.bfloat16
    fp8 = ml_dtypes.float8_e4m3

    counts = np.bincount(label, minlength=C)

    # greedy: biggest class -> least-loaded core (cap 128 classes/core)
    order_cls = np.argsort(-counts, kind="stable")
    core_load = np.zeros(N_CORES, dtype=np.int64)
    core_ncls = np.zeros(N_CORES, dtype=np.int64)
    cls_lists = [[] for _ in range(N_CORES)]
    nblk = (counts + GTOK - 1) // GTOK  # blocks per class
    for c in order_cls:
        k = min((k for k in range(N_CORES) if core_ncls[k] < P),
                key=lambda k: core_load[k])
        cls_lists[k].append(c)
        core_load[k] += nblk[c]
        core_ncls[k] += 1

    T = int(max(1, -(-core_load.max() // P)))
    cap_blk = T * P
    cap_tok = cap_blk * GTOK

    sort_order = np.argsort(label, kind="stable")
    starts = np.zeros(C + 1, dtype=np.int64)
    np.cumsum(counts, out=starts[1:])

    feat_bf = np.ascontiguousarray(feature, dtype=np.float32).astype(bf16)

    src_all = np.full(N_CORES * cap_tok, -1, dtype=np.int64)
    metas = []
    for k in range(N_CORES):
        base = k * cap_tok
        pos = 0
        mcls = np.full(cap_blk, -1, dtype=np.int64)
        mcnt = np.zeros(cap_blk, dtype=np.float32)
        blk = 0
        for li, c in enumerate(cls_lists[k]):
            ncv = int(counts[c])
            if ncv:
                src_all[base + pos: base + pos + ncv] = \
                    sort_order[starts[c]: starts[c] + ncv]
            nb = int(nblk[c])
            if nb:
                mcls[blk: blk + nb] = li
                mcnt[blk: blk + nb] = GTOK
                mcnt[blk + nb - 1] = ncv - (nb - 1) * GTOK
            pos += nb * GTOK
            blk += nb
        metas.append((mcls, mcnt))

    nblk_tot = N_CORES * cap_blk
    valid = src_all >= 0
    tok = np.zeros((nblk_tot * GTOK, D), dtype=bf16)
    tok[valid] = feat_bf[src_all[valid]]
    arr = tok.reshape(N_CORES, T * P, FW)

    proto32 = np.ascontiguousarray(prototype, dtype=np.float32)
    in_maps = []
    for k in range(N_CORES):
        cl = np.asarray(cls_lists[k], dtype=np.int64)
        pk = np.zeros((P, D), dtype=np.float32)
        pk[: len(cl)] = proto32[cl]
        # one-hot row per block (zero row for pad blocks)
        mcls = metas[k][0]
        ohh = np.zeros((cap_blk, P), dtype=bf16)
        real = mcls >= 0
        ohh[np.flatnonzero(real), mcls[real]] = 1
        # block b=(tile j, partition p) -> blk_cnt[p, j]
        meta2_k = np.ascontiguousarray(
            metas[k][1].reshape(T, P).T.astype(bf16))
        in_maps.append({
            "feature": np.ascontiguousarray(arr[k]),
            "oh_rows": ohh,
            "blk_cnt": meta2_k,
            "prototype": pk,
        })
    return in_maps, cls_lists, T


_NC_CACHE = {}


def run(inputs: dict, trace: bool = False):
    feature = np.asarray(inputs["feature"])
    label = np.asarray(inputs["label"], dtype=np.int64)
    prototype = np.asarray(inputs["prototype"])
    step = int(np.asarray(inputs["step"]))

    in_maps, cls_lists, T = shard_inputs(feature, label, prototype)
    key = (T, step > WARMUP_STEP)
    if key not in _NC_CACHE:
        _NC_CACHE[key] = build_nc(T, step > WARMUP_STEP)
    nc = _NC_CACHE[key]
    res = bass_utils.run_bass_kernel_spmd(
        nc, in_maps, core_ids=list(range(N_CORES)), trace=trace,
    )
    out = np.ascontiguousarray(prototype, dtype=np.float32).copy()
    for k in range(N_CORES):
        cl = np.asarray(cls_lists[k], dtype=np.int64)
        ok = np.asarray(res.results[k]["out"], dtype=np.float32)
        out[cl] = ok[: len(cl)]
    return out, res


def kernel(**inputs) -> np.ndarray:
    out, _ = run(inputs, trace=False)
    return out
